# revision 31
# baseline (speedup 1.0000x reference)
import sys, os
sys.path.insert(0, "/opt/trn_rl_repo")
import numpy as np

import concourse.bass as bass
import concourse.tile as tile
from concourse import mybir
from concourse.bass_utils import run_bass_kernel_spmd

F32 = mybir.dt.float32
AF = mybir.ActivationFunctionType
ALU = mybir.AluOpType

B, W, M, F = 32, 16, 256, 16
H = 64
HALF = 32
K = 10
NCORES = 8
BL = B // NCORES          # 4 batches per core
N = BL * M                # 1024 sequences per core


def _host_prep(inputs):
    """Precompute all weight layouts on host (f32 contiguous)."""
    g = {k: np.asarray(v, dtype=np.float32) for k, v in inputs.items()}
    Wih, Whh, bih, bhh = g['Wih'], g['Whh'], g['bih'], g['bhh']
    V, W1, W2, b1 = g['V'], g['W1'], g['W2'], g['b1']
    d = {}
    c = np.ascontiguousarray
    d['whhT_r'] = c(Whh[0:64].T);   d['whhT_z'] = c(Whh[64:128].T)
    d['whhT_n'] = c(Whh[128:192].T)
    d['wihT_r'] = c(Wih[0:64].T);   d['wihT_z'] = c(Wih[64:128].T)
    d['wihT_n'] = c(Wih[128:192].T)
    d['br2'] = c(np.tile(bih[0:64] + bhh[0:64], 2)[:, None])
    d['bz2'] = c(np.tile(bih[64:128] + bhh[64:128], 2)[:, None])
    d['bin2'] = c(np.tile(bih[128:192], 2)[:, None])
    d['bhh_n2'] = c(np.tile(bhh[128:192], 2)[None, :])
    d['i64'] = np.eye(64, dtype=np.float32)
    d['w1T'] = c(W1.T)                       # (64, 32)
    d['w2T4'] = c(np.tile(W2.T, (1, 4)))     # (64, 128)
    d['v32'] = c(V[:, None])                 # (32, 1)
    d['b1rep4'] = c(np.tile(b1, 4)[:, None])  # (128,1)
    d['b1col'] = c(b1[:, None])              # (32,1)
    # V block lhsT variants: vp[:, 32*r + m], nonzero at m=4r+c row 32c+hh
    vp = np.zeros((128, 256), dtype=np.float32)
    for r in range(8):
        for cc in range(4):
            for hh in range(32):
                vp[32 * cc + hh, 32 * r + 4 * r + cc] = V[hh]
    d['vp'] = vp
    d['vn'] = -vp
    d['cwT'] = c(np.transpose(g['conv_w'], (1, 2, 0)).reshape(16, 160))
    d['clT'] = c(np.transpose(g['convl_w'], (1, 2, 0)).reshape(16, 80))
    d['rbcol'] = c(g['conv_b'][:, None])
    d['rlbcol'] = c(g['convl_b'][:, None])
    d['g1a'] = c(g['gc1_w'][0::3]); d['g1b'] = c(g['gc1_w'][1::3])
    d['g1c'] = c(g['gc1_w'][2::3])
    d['gc1bcol'] = c(g['gc1_b'][:, None])
    d['gc2w'] = c(g['gc2_w'])
    d['gc2bcol'] = c(g['gc2_b'][:, None])
    d['ow_sp'] = c(g['out_w'][0, 0:10][:, None])
    d['ow_h'] = c(g['out_w'][0, 10:74][:, None])
    d['adjT'] = c(g['adj'].T)
    d['Wb'] = c(g['Wb'])
    d['id128'] = np.eye(128, dtype=np.float32)
    d['ones128'] = np.ones((1, 128), dtype=np.float32)
    d['onesN'] = np.ones((1, 1024), dtype=np.float32)
    d['z128t'] = np.zeros((128, 1), dtype=np.float32)
    d['wbscolt'] = np.full((128, 1), float(g['wb'][0]), dtype=np.float32)
    d['obcolt'] = np.full((1, 1), float(g['out_b'][0]), dtype=np.float32)
    d['z1t'] = np.zeros((1, 1), dtype=np.float32)
    for k in ('whhT_r', 'whhT_z', 'whhT_n', 'i64', 'w2T4', 'w1T', 'ow_h'):
        d[k] = np.ascontiguousarray(np.concatenate([d[k], d[k]], axis=0))
    consts = dict(
        ob=float(g['out_b'][0]), wbs=float(g['wb'][0]),
        cvs=float(g['bv'][0] + float(V @ b1) - float(V.sum())),
    )
    return d, consts


WNAMES = ['whhT_r', 'whhT_z', 'whhT_n', 'wihT_r', 'wihT_z', 'wihT_n',
          'br2', 'bz2', 'bin2', 'bhh_n2', 'i64', 'w1T', 'w2T4', 'v32',
          'b1rep4', 'b1col', 'vp', 'vn', 'cwT', 'clT', 'rbcol', 'rlbcol',
          'g1a', 'g1b', 'g1c', 'gc1bcol', 'gc2w', 'gc2bcol', 'ow_sp',
          'ow_h', 'adjT', 'Wb', 'id128', 'ones128', 'onesN', 'z128t', 'wbscolt', 'obcolt', 'z1t']


def build_nc(wshapes, consts):
    nc = bass.Bass("TRN2", target_bir_lowering=False, debug=False)
    X_d = nc.dram_tensor("X", [BL, W, M, F], F32, kind="ExternalInput")
    Y_d = nc.dram_tensor("Yv", [1, N], F32, kind="ExternalInput")
    wd = {k: nc.dram_tensor(k, list(wshapes[k]), F32, kind="ExternalInput")
          for k in WNAMES}
    probs_d = nc.dram_tensor("probs", [1, N], F32, kind="ExternalOutput")
    loss_d = nc.dram_tensor("lossp", [1, 1], F32, kind="ExternalOutput")

    ob, wbs, cvs = consts['ob'], consts['wbs'], consts['cvs']

    def MM(*a, **k):
        k.setdefault('skip_group_check', True)
        return nc.tensor.matmul(*a, **k)

    def pst(pool, shape, tag):
        t = pool.tile(shape, F32, tag=tag)
        nc.vector.memset(t[0:1, 0:1], 0.0)
        return t

    with tile.TileContext(nc) as tc:
        from contextlib import ExitStack
        with ExitStack() as ctx:
            cp = ctx.enter_context(tc.tile_pool(name="consts", bufs=1))
            sp = ctx.enter_context(tc.tile_pool(name="state", bufs=1))
            wp = ctx.enter_context(tc.tile_pool(name="work", bufs=1))
            up = ctx.enter_context(tc.tile_pool(name="uchunks", bufs=2))
            pg = ctx.enter_context(tc.tile_pool(name="psg", bufs=1, space="PSUM"))
            pgn = ctx.enter_context(tc.tile_pool(name="psn", bufs=1, space="PSUM"))
            pa = ctx.enter_context(tc.tile_pool(name="psa", bufs=1, space="PSUM"))
            ps = ctx.enter_context(tc.tile_pool(name="pss", bufs=2, space="PSUM"))

            # ---- load weights ----
            w = {}
            DUPK = ('whhT_r', 'whhT_z', 'whhT_n', 'i64', 'w2T4', 'w1T', 'ow_h')
            MMFEED = set(DUPK) | {'wihT_r', 'wihT_z', 'wihT_n', 'bhh_n2', 'v32',
                                  'vp', 'vn', 'cwT', 'clT', 'g1a', 'g1b', 'g1c',
                                  'gc2w', 'ow_sp', 'Wb', 'ones128', 'onesN', 'id128'}
            def _load(k, shape, src_ap):
                raw = cp.tile(shape, F32, tag=f"{k}_r")
                nc.sync.dma_start(raw[:], src_ap)
                if k.rstrip('01') in MMFEED or k in MMFEED:
                    t = cp.tile(shape, F32, tag=k)
                    nc.vector.tensor_copy(t[:], raw[:])
                    return t
                return raw
            for k in WNAMES:
                if k in DUPK:
                    w[k] = _load(k, [128, wshapes[k][1]], wd[k].ap())
                    continue
                if k in ('adjT', 'Wb'):
                    w[k] = [_load(f"{k}{hh_}" if k == 'adjT' else k + str(hh_),
                                  [128, 256], wd[k].ap()[128 * hh_:128 * (hh_ + 1), :])
                            for hh_ in range(2)]
                    continue
                w[k] = _load(k, list(wshapes[k]), wd[k].ap())
            y_sb = cp.tile([1, N], F32, tag="y")
            nc.sync.dma_start(y_sb[:], Y_d.ap())

            # ---- xt tiles: (16 f, N) per timestep w, strided DMA ----
            xts = []
            xsl = []
            for mt in range(2):
                xm = up.tile([128, 1024], F32, tag="U")
                xs_ = cp.tile([128, 1024], F32, tag=f"xs{mt}")
                for bb in range(BL):
                    src = X_d.ap()[bb, :, 128 * mt:128 * (mt + 1), :].rearrange("w m f -> m w f")
                    nc.sync.dma_start(
                        xm[:, 256 * bb:256 * (bb + 1)].rearrange("m (w f) -> m w f", w=W), src)
                    nc.vector.tensor_copy(xs_[:, 256 * bb:256 * (bb + 1)],
                                          xm[:, 256 * bb:256 * (bb + 1)])
                xsl.append(xs_)
            for t in range(W):
                xt = cp.tile([F, N], F32, tag=f"xt{t}")
                for half in range(2):
                    pt_ = pst(ps, [F, 512], "sm")
                    for mt in range(2):
                        for bb in (2 * half, 2 * half + 1):
                            MM(pt_[:, 256 * (bb - 2 * half) + 128 * mt:256 * (bb - 2 * half) + 128 * mt + 128],
                               xsl[mt][:, 256 * bb + 16 * t:256 * bb + 16 * t + 16],
                               w['id128'][:], is_transpose=True)
                    nc.vector.tensor_copy(xt[:, 512 * half:512 * (half + 1)], pt_[:])
                xts.append(xt)

            ones_t = w['onesN']
            z128 = w['z128t']
            wbscol = w['wbscolt']
            obcol = w['obcolt']
            z1 = w['z1t']

            # ---- conv branch (independent of GRU) ----
            convs = []
            for name, pairs, bias in (
                ("rsh", [(t, w['cwT'][:, 10 * t:10 * t + 10]) for t in range(16)], 'rbcol'),
                ("rl0", [(2 * j, w['clT'][:, 10 * j:10 * j + 10]) for j in range(8)], 'rlbcol'),
                ("rl1", [(2 * j + 1, w['clT'][:, 10 * j:10 * j + 10]) for j in range(8)], 'rlbcol'),
            ):
                pc = pst(pg, [K, N], "big")
                for i, (t, lhsT) in enumerate(pairs):
                    for hf in range(2):
                        MM(pc[:, 512 * hf:512 * (hf + 1)], lhsT,
                                         xts[t][:, 512 * hf:512 * (hf + 1)],
                                         start=(i == 0), stop=(i == len(pairs) - 1))
                rtU = up.tile([128, 1024], F32, tag="U")
                rt0 = rtU[0:K, :]
                nc.scalar.activation(rt0[:], pc[:], AF.Relu, bias=w[bias][:])
                rt = cp.tile([K, N], F32, tag=name)
                nc.vector.tensor_copy(rt[:], rt0[:])
                convs.append(rt)
            rsh, rl0, rl1 = convs

            # ---- GRU ----
            h = sp.tile([128, 512], F32, tag="h")    # rows 0-63: n 0-511; 64-127: n 512-1023
            nc.vector.memset(h[:], 0.0)
            for t in range(W):
                xt = xts[t]
                prz = pst(pg, [128, 1024], "big")   # cols 0-511 r | 512-1023 z
                pn = pst(pgn, [128, 512], "pn")
                pin = pst(pgn, [128, 512], "pin")
                # r / z  (h-side + x-side), packed rows A|B
                MM(prz[0:64, 0:512], w['whhT_r'][0:64, :], h[0:64, :], start=True, stop=False)
                MM(prz[64:128, 0:512], w['whhT_r'][64:128, :], h[64:128, :], start=True, stop=False, tile_position=(64, 64))
                MM(prz[0:64, 512:1024], w['whhT_z'][0:64, :], h[0:64, :], start=True, stop=False)
                MM(prz[64:128, 512:1024], w['whhT_z'][64:128, :], h[64:128, :], start=True, stop=False, tile_position=(64, 64))
                MM(prz[0:64, 0:512], w['wihT_r'][:], xt[:, 0:512], start=False, stop=False)
                MM(prz[64:128, 0:512], w['wihT_r'][:], xt[:, 512:1024], start=False, stop=True, tile_position=(0, 64))
                MM(prz[0:64, 512:1024], w['wihT_z'][:], xt[:, 0:512], start=False, stop=False)
                MM(prz[64:128, 512:1024], w['wihT_z'][:], xt[:, 512:1024], start=False, stop=True, tile_position=(0, 64))
                # n-gate h-side with bhh bias;  x-side separate psum
                MM(pn[:, :], w['bhh_n2'][:], ones_t[:, 0:512], start=True, stop=False)
                MM(pn[0:64, :], w['whhT_n'][0:64, :], h[0:64, :], start=False, stop=False)
                MM(pn[64:128, :], w['whhT_n'][64:128, :], h[64:128, :], start=False, stop=True, tile_position=(64, 64))
                MM(pin[0:64, :], w['wihT_n'][:], xt[:, 0:512], start=True, stop=False)
                MM(pin[64:128, :], w['wihT_n'][:], xt[:, 512:1024], start=True, stop=False, tile_position=(0, 64))
                gates = wp.tile([128, 1024], F32, tag="gates")
                nc.scalar.activation(gates[:, 0:512], prz[:, 0:512], AF.Sigmoid, bias=w['br2'][:])
                nc.scalar.activation(gates[:, 512:1024], prz[:, 512:1024], AF.Sigmoid, bias=w['bz2'][:])
                tmp = wp.tile([128, 512], F32, tag="tmp")
                nc.vector.tensor_mul(tmp[:], gates[:, 0:512], pn[:])
                MM(pin[0:64, :], w['i64'][0:64, :], tmp[0:64, :], start=False, stop=False)
                MM(pin[64:128, :], w['i64'][64:128, :], tmp[64:128, :], start=False, stop=True, tile_position=(64, 64))
                n_sb = wp.tile([128, 512], F32, tag="nsb")
                nc.scalar.activation(n_sb[:], pin[:], AF.Tanh, bias=w['bin2'][:])
                dt_ = wp.tile([128, 512], F32, tag="dt")
                nc.vector.tensor_sub(dt_[:], h[:], n_sb[:])
                et = wp.tile([128, 512], F32, tag="et")
                nc.vector.tensor_mul(et[:], gates[:, 512:1024], dt_[:])
                nc.vector.tensor_add(h[:], n_sb[:], et[:])

            def hsl(b):
                if b < 2:
                    return h[0:64, 256 * b:256 * (b + 1)], 0
                return h[64:128, 256 * (b - 2):256 * (b - 1)], 64

            # ---- attention ----
            a_n = {}
            nrm2 = {}
            for b in range(BL):
                hb, hbase = hsl(b)
                p2 = pst(ps, [128, 256], "sm")
                MM(p2[:], w['w2T4'][hbase:hbase + 64, :], hb, start=True, stop=True,
                                 tile_position=(hbase, 0))
                s2rep = wp.tile([128, 256], F32, tag="s2rep")
                nc.vector.tensor_copy(s2rep[:], p2[:])
                p1 = pst(ps, [128, 64], "sm")
                for cc in range(4):
                    MM(p1[32 * cc:32 * cc + 32, :], w['w1T'][hbase:hbase + 64, :],
                                     hb.rearrange("h (t c) -> h t c", c=4)[:, :, cc],
                                     start=True, stop=True, tile_position=(hbase, 32 * cc))
                s1b = wp.tile([128, 64], F32, tag="s1b")
                nc.vector.tensor_scalar(s1b[:], p1[:], w['b1rep4'][:], None, op0=ALU.add)
                p1t = pst(ps, [32, 256], "sm")
                MM(p1t[:], w['w1T'][hbase:hbase + 64, :], hb, start=True, stop=True,
                                 tile_position=(hbase, 0))
                s1T = wp.tile([32, 256], F32, tag="s1T")
                nc.vector.tensor_scalar(s1T[:], p1t[:], w['b1col'][:], None, op0=ALU.add)
                pv1 = pst(ps, [1, 256], "sm")
                MM(pv1[:], w['v32'][:], s1T[:], start=True, stop=True)
                vs1 = wp.tile([1, 256], F32, tag="vs1")
                nc.vector.tensor_scalar(vs1[:], pv1[:], cvs, None, op0=ALU.add)
                pv2 = pst(ps, [1, 256], "sm")
                MM(pv2[:], w['v32'][:], s2rep[0:32, :], start=True, stop=True)
                vs2 = wp.tile([1, 256], F32, tag="vs2")
                nc.vector.tensor_copy(vs2[:], pv2[:])

                for pt in range(2):
                    pa_t = pst(pa, [128, 256], "pa")
                    MM(pa_t[:], vs1[:, 128 * pt:128 * pt + 128], ones_t[:, 0:256],
                                     start=True, stop=False)
                    MM(pa_t[:], w['ones128'][:], vs2[:], start=False, stop=False)
                    for chunk in range(8):
                        U = up.tile([128, 1024], F32, tag="U")
                        for s in range(4):
                            idx = chunk * 4 + s
                            gg, rr = idx // 8, idx % 8
                            tcol = 32 * pt + 8 * gg + rr
                            nc.vector.tensor_scalar(
                                U[:, 256 * s:256 * (s + 1)], s2rep[:],
                                s1b[:, tcol:tcol + 1], 0.0, op0=ALU.add, op1=ALU.min)
                        for s in range(4):
                            idx = chunk * 4 + s
                            gg, rr = idx // 8, idx % 8
                            MM(pa_t[32 * gg:32 * gg + 32, :],
                                             w['vn'][:, 32 * rr:32 * rr + 32],
                                             U[:, 256 * s:256 * (s + 1)],
                                             start=False, stop=False, tile_position=(0, 32 * gg))
                        nc.scalar.activation(U[:], U[:], AF.Exp, bias=z128[:])
                        for s in range(4):
                            idx = chunk * 4 + s
                            gg, rr = idx // 8, idx % 8
                            last = (chunk == 7 and s == 3)
                            MM(pa_t[32 * gg:32 * gg + 32, :],
                                             w['vp'][:, 32 * rr:32 * rr + 32],
                                             U[:, 256 * s:256 * (s + 1)],
                                             start=False, stop=last, tile_position=(0, 32 * gg))
                    a_sb = cp.tile([128, 256], F32, tag=f"a{b}_{pt}")
                    nc.vector.tensor_copy(a_sb[:], pa_t[:])
                    sq = wp.tile([128, 256], F32, tag="sq")
                    n2 = cp.tile([128, 1], F32, tag=f"n2_{b}_{pt}")
                    nc.scalar.activation(sq[:], a_sb[:], AF.Square, bias=z128[:], accum_out=n2[:])
                    a_n[(b, pt)] = a_sb
                    nrm2[(b, pt)] = n2

            # ---- normalize columns ----
            for b in range(BL):
                for pt in range(2):
                    nrm = wp.tile([128, 1], F32, tag="nrm")
                    nc.scalar.activation(nrm[:], nrm2[(b, pt)][:], AF.Sqrt, bias=z128[:])
                    nc.vector.tensor_scalar(nrm[:], nrm[:], 1e-12, None, op0=ALU.max)
                    rin = wp.tile([128, 1], F32, tag="rin")
                    nc.vector.reciprocal(rin[:], nrm[:])
                    nc.vector.tensor_scalar(a_n[(b, pt)][:], a_n[(b, pt)][:], rin[:],
                                            None, op0=ALU.mult)

            # ---- gated adjacency + GCN ----
            AT = {}
            for b in range(BL):
                for jt in range(2):
                    pc2 = pst(ps, [128, 256], "sm")
                    for kt in range(2):
                        MM(pc2[:], w['Wb'][kt][:, 128 * jt:128 * jt + 128],
                                         a_n[(b, kt)][:], start=(kt == 0), stop=(kt == 1))
                    cT = wp.tile([128, 256], F32, tag="cT")
                    nc.scalar.activation(cT[:], pc2[:], AF.Sigmoid, bias=wbscol[:])
                    dl = wp.tile([128, 256], F32, tag="dl")
                    nc.vector.tensor_sub(dl[:], w['adjT'][jt][:], a_n[(b, jt)][:])
                    el = wp.tile([128, 256], F32, tag="el")
                    nc.vector.tensor_mul(el[:], cT[:], dl[:])
                    at = cp.tile([128, 256], F32, tag=f"AT{b}_{jt}")
                    nc.vector.tensor_add(at[:], a_n[(b, jt)][:], el[:])
                    AT[(b, jt)] = at

            logits_sb = sp.tile([1, N], F32, tag="logits")
            for b in range(BL):
                h1sb = {}
                for nt in range(2):
                    ph1 = pst(ps, [128, 64], "sm")
                    col = 256 * b + 128 * nt
                    MM(ph1[:], rsh[:, col:col + 128], w['g1a'][:], start=True, stop=False)
                    MM(ph1[:], rl0[:, col:col + 128], w['g1b'][:], start=False, stop=False)
                    MM(ph1[:], rl1[:, col:col + 128], w['g1c'][:], start=False, stop=True)
                    h1 = wp.tile([128, 64], F32, tag=f"h1_{nt}")
                    nc.vector.tensor_copy(h1[:], ph1[:])
                    h1sb[nt] = h1
                px1 = pst(ps, [64, 256], "sm")
                for jt in range(2):
                    MM(px1[:], h1sb[jt][:], AT[(b, jt)][:], start=(jt == 0), stop=(jt == 1))
                x1Ta = wp.tile([64, 256], F32, tag="x1Ta")
                nc.scalar.activation(x1Ta[:], px1[:], AF.Relu, bias=w['gc1bcol'][:])
                x1T = wp.tile([64, 256], F32, tag="x1T")
                nc.vector.tensor_copy(x1T[:], x1Ta[:])
                h2sb = {}
                for nt in range(2):
                    ph2 = pst(ps, [128, 10], "sm")
                    MM(ph2[:], x1T[:, 128 * nt:128 * nt + 128], w['gc2w'][:], start=True, stop=True)
                    h2 = wp.tile([128, 10], F32, tag=f"h2_{nt}")
                    nc.vector.tensor_copy(h2[:], ph2[:])
                    h2sb[nt] = h2
                psp = pst(ps, [10, 256], "sm")
                for jt in range(2):
                    MM(psp[:], h2sb[jt][:], AT[(b, jt)][:], start=(jt == 0), stop=(jt == 1))
                spTa = wp.tile([10, 256], F32, tag="spTa")
                nc.scalar.activation(spTa[:], psp[:], AF.Relu, bias=w['gc2bcol'][:])
                spT = wp.tile([10, 256], F32, tag="spT")
                nc.vector.tensor_copy(spT[:], spTa[:])
                hb, hbase = hsl(b)
                lpb = pst(ps, [1, 256], "sm")
                MM(lpb[:], w['ow_sp'][:], spT[:], start=True, stop=False)
                MM(lpb[:], w['ow_h'][hbase:hbase + 64, :], hb, start=False, stop=True,
                                 tile_position=(hbase, 0))
                nc.vector.tensor_copy(logits_sb[:, 256 * b:256 * (b + 1)], lpb[:])

            # ---- outputs ----
            probs_sb = sp.tile([1, N], F32, tag="probs")
            nc.scalar.activation(probs_sb[:], logits_sb[:], AF.Sigmoid, bias=obcol[:])
            nc.sync.dma_start(probs_d.ap(), probs_sb[:])
            t1 = wp.tile([1, N], F32, tag="t1")
            nc.scalar.activation(t1[:], logits_sb[:], AF.Relu, bias=obcol[:])
            labs = wp.tile([1, N], F32, tag="labs")
            nc.scalar.activation(labs[:], logits_sb[:], AF.Abs, bias=obcol[:])
            enl = wp.tile([1, N], F32, tag="enl")
            nc.scalar.activation(enl[:], labs[:], AF.Exp, bias=z1[:], scale=-1.0)
            nc.vector.tensor_scalar(enl[:], enl[:], 1.0, None, op0=ALU.add)
            sp_ = wp.tile([1, N], F32, tag="sp_")
            nc.scalar.activation(sp_[:], enl[:], AF.Ln, bias=z1[:])
            lsb = wp.tile([1, N], F32, tag="lsb")
            nc.vector.tensor_scalar(lsb[:], logits_sb[:], ob, None, op0=ALU.add)
            ly = wp.tile([1, N], F32, tag="ly")
            nc.vector.tensor_mul(ly[:], lsb[:], y_sb[:])
            acc = wp.tile([1, N], F32, tag="acc")
            nc.vector.tensor_sub(acc[:], t1[:], ly[:])
            nc.vector.tensor_add(acc[:], acc[:], sp_[:])
            lout = sp.tile([1, 1], F32, tag="lout")
            nc.vector.reduce_sum(lout[:], acc[:], axis=mybir.AxisListType.X)
            nc.sync.dma_start(loss_d.ap(), lout[:])
    _split_multi_waits(nc)
    return nc


def _split_multi_waits(nc):
    """This container's walrus accepts only one sync wait per instruction.
    Hoist extra waits onto standalone EventSemaphore (wait) instructions."""
    cnt = 0
    for f in nc.m.functions:
        for bb in f.blocks:
            out = []
            for inst in bb.instructions:
                si = getattr(inst, 'sync_info', None)
                ws = list(si.on_wait) if (si is not None and si.on_wait) else []
                if len(ws) > 1:
                    for j, wc in enumerate(ws[:-1]):
                        n = mybir.InstEventSemaphore(
                            name=f"{inst.name}_hw{j}",
                            ins=[], outs=[],
                        )
                        n.engine = inst.engine
                        n.sync_info = mybir.SyncInfo(on_wait=[wc], on_update=[])
                        out.append(n)
                        cnt += 1
                    si.on_wait = [ws[-1]]
                out.append(inst)
            bb.instructions = out
    print(f"[split_multi_waits] hoisted {cnt} waits")


_CACHE = {}


def _get_nc(wshapes, consts):
    key = "nc"
    if key not in _CACHE:
        _CACHE[key] = build_nc(wshapes, consts)
    return _CACHE[key]


def kernel(**inputs):
    d, consts = _host_prep(inputs)
    wshapes = {k: d[k].shape for k in WNAMES}
    nc = _get_nc(wshapes, consts)
    X = np.asarray(inputs['X'], dtype=np.float32)
    Y = np.asarray(inputs['Y'], dtype=np.float32)
    in_maps = []
    for c in range(NCORES):
        m = {k: d[k] for k in WNAMES}
        m['X'] = np.ascontiguousarray(X[BL * c:BL * (c + 1)])
        m['Yv'] = np.ascontiguousarray(Y[BL * c:BL * (c + 1)].reshape(1, N))
        in_maps.append(m)
    res = run_bass_kernel_spmd(nc, in_maps, list(range(NCORES)))
    if res.exec_time_ns is not None:
        print(f"HW exec time: {res.exec_time_ns} ns")
    probs = np.concatenate([res.results[c]['probs'].reshape(-1) for c in range(NCORES)])
    loss = sum(float(res.results[c]['lossp'].reshape(-1)[0]) for c in range(NCORES)) / (B * M)
    return np.float32(loss), probs.astype(np.float32)
